# revision 19
# baseline (speedup 1.0000x reference)
"""Multi-head attention (B=4, S=2048, H=1024, NH=16) on 8 trn2 NeuronCores.

Sharding: token-parallel, no collectives. Core c handles batch b=c//2,
query half h=c%2 (1024 query tokens), with the full 2048-key K/V of its
batch (K/V projection duplicated within each core pair).

Per-core pipeline v2 (row-tiled scores, fp8 probabilities):
  A) Q projection -> persistent SBUF slabs Qp[og] [128, TOK] bf16 (head
     pair 2og/2og+1 in row halves, no zero padding).  K projection ->
     feature-major slabs spilled to DRAM (og0 kept in SBUF).  V
     projection -> SBUF token-major Vt [tok, 16*65] with per-head ones
     column (PV then also yields the softmax denominator), emitted in
     512/512/16-column parts.  1/8 attention scale folded into Wq.
  B) Attention per head PAIR: scoresT[k,q] for both heads run
     CONCURRENTLY on the PE via K=64 row tiling (rows 0-63 = even head,
     64-127 = odd head) into separate PSUM tiles; exp on ScalarE
     (attention mask + a -2.0 overflow guard as per-partition bias)
     emits fp8e4 E tiles (softmax shift-invariance makes the guard
     free); PV ctx[q, 65] accumulated over 16 key chunks from fp8 E
     (fp8 stationary also gets 4x FWL weight loads), normalized by the
     ones-column denominator into per-pair ctx chunks [128 tok, 128].
  C) Output projection is spread across pairs: as soon as pair p's PV
     completes, its ctx chunk is PE-transposed and matmul'd against
     Wo rows 128p..128p+127, accumulated into SBUF O_partial via DVE;
     the final pair adds the bias and streams the result out, so only
     ~1/8 of the O projection remains after the last exp.
"""

import numpy as np
import ml_dtypes

import concourse.tile as tile
from concourse import bacc, mybir
from concourse.bass_utils import run_bass_kernel_spmd
from concourse.masks import make_identity

B, S, H, NH, HDIM = 4, 2048, 1024, 16, 64
NCORES = 8
TOK = 1024            # query tokens per core
KTOK = 2048           # key tokens per core
IC = H // 128         # 8 feature chunks of 128
KC = KTOK // 128      # 16 key chunks of 128
QT = TOK // 128       # 8 query tiles of 128
NP = IC               # 8 head pairs
VW = NH * (HDIM + 1)  # 1040: V columns incl. per-head ones column
BF = mybir.dt.bfloat16
F32 = mybir.dt.float32
E_BUFS = 20

_CACHE = {}


def _emit(nc, tc, io):
    Exp = mybir.ActivationFunctionType.Exp
    Alu = mybir.AluOpType

    persist = tc.alloc_tile_pool(name="persist", bufs=1)
    psum = tc.alloc_tile_pool(name="psum", bufs=2, space="PSUM")
    attnp = tc.alloc_tile_pool(name="attnp", bufs=1)

    kt_store = nc.dram_tensor("kt_store", [IC, 128, KTOK], BF).ap()

    # ---- persistent tiles ----
    ident = persist.tile([128, 128], BF, name="ident", tag="ident")
    make_identity(nc, ident[:])
    mask_sb = persist.tile([128, KC], F32, name="mask_sb", tag="mask_sb")
    nc.sync.dma_start(mask_sb[:], io["maskcol"][:])
    bqc = persist.tile([128, IC], F32, name="bqc", tag="bqc")
    nc.sync.dma_start(bqc[:], io["bqcol"][:])
    bkc = persist.tile([128, IC], F32, name="bkc", tag="bkc")
    nc.sync.dma_start(bkc[:], io["bkcol"][:])

    Qp = [persist.tile([128, TOK], BF, name=f"Qp{i}", tag=f"Qp{i}")
          for i in range(IC)]
    Vt = [persist.tile([128, VW], BF, name=f"Vt{i}", tag=f"Vt{i}")
          for i in range(KC)]

    # ---- Q projection (og granular) into persistent SBUF slabs ----
    q_pool = {}

    def open_q_pool():
        ap = tc.alloc_tile_pool(name="q_pool", bufs=1, side="right")
        q_pool["pool"] = ap
        q_pool["x"] = []
        q_pool["w"] = []
        for i in range(IC):
            x = ap.tile([128, TOK], BF, name=f"q_x{i}", tag=f"qx{i}")
            nc.sync.dma_start(x[:], io["qT"][i * 128:(i + 1) * 128, :])
            q_pool["x"].append(x)
            w = ap.tile([128, H], BF, name=f"q_w{i}", tag=f"qw{i}")
            nc.sync.dma_start(w[:, 0:512],
                              io["wqT"][i * 128:(i + 1) * 128, 0:512])
            q_pool["w"].append(w)
        for i in range(IC):
            nc.sync.dma_start(q_pool["w"][i][:, 512:1024],
                              io["wqT"][i * 128:(i + 1) * 128, 512:1024])

    def emit_q_og(og):
        x_s, w_s = q_pool["x"], q_pool["w"]
        pa = psum.tile([128, 512], F32, name="ps_qa", tag="proj")
        pb = psum.tile([128, 512], F32, name="ps_qb", tag="proj")
        for i in range(IC):
            w = w_s[i][:, og * 128:(og + 1) * 128]
            nc.tensor.matmul(pa[:], w, x_s[i][:, 0:512],
                             start=(i == 0), stop=(i == IC - 1))
            nc.tensor.matmul(pb[:], w, x_s[i][:, 512:1024],
                             start=(i == 0), stop=(i == IC - 1))
        nc.vector.tensor_scalar_add(Qp[og][:, 0:512], pa[:], bqc[:, og:og + 1])
        nc.vector.tensor_scalar_add(Qp[og][:, 512:1024], pb[:],
                                    bqc[:, og:og + 1])

    # ---- K projection: feature-major slabs to DRAM (og0 in SBUF) ----
    k_pool = {}

    def open_k_pool():
        ap = tc.alloc_tile_pool(name="k_pool", bufs=1)
        k_pool["pool"] = ap
        k_pool["w"] = []
        k_pool["x"] = []
        for i in range(IC):
            w = ap.tile([128, H], BF, name=f"k_w{i}", tag=f"kw{i}", bufs=1)
            nc.sync.dma_start(w[:, 0:512],
                              io["wkT"][i * 128:(i + 1) * 128, 0:512])
            k_pool["w"].append(w)
            x = ap.tile([128, KTOK], BF, name=f"k_x{i}", tag=f"kx{i}", bufs=1)
            nc.sync.dma_start(x[:], io["kT"][i * 128:(i + 1) * 128, :])
            k_pool["x"].append(x)
        for i in range(IC):
            nc.sync.dma_start(k_pool["w"][i][:, 512:1024],
                              io["wkT"][i * 128:(i + 1) * 128, 512:1024])

    def emit_k_og(og, tps):
        ap, w_s, x_s = k_pool["pool"], k_pool["w"], k_pool["x"]
        for tp in tps:
            pa = psum.tile([128, 512], F32, name="ps_ka", tag="proj")
            pb = psum.tile([128, 512], F32, name="ps_kb", tag="proj")
            for i in range(IC):
                w = w_s[i][:, og * 128:(og + 1) * 128]
                nc.tensor.matmul(pa[:], w, x_s[i][:, tp * 1024:tp * 1024 + 512],
                                 start=(i == 0), stop=(i == IC - 1))
                nc.tensor.matmul(pb[:], w,
                                 x_s[i][:, tp * 1024 + 512:tp * 1024 + 1024],
                                 start=(i == 0), stop=(i == IC - 1))
            for ps, tg in ((pa, 0), (pb, 1)):
                col = tp * 1024 + tg * 512
                se = ap.tile([128, 512], BF, name="k_se", tag="kse", bufs=1)
                nc.vector.tensor_scalar_add(se[:], ps[:], bkc[:, og:og + 1])
                nc.sync.dma_start(kt_store[og][:, col:col + 512], se[:])

    # ---- V projection: token-major Vt, 512/512/16 column parts ----
    v_pool = {}

    def open_v_pool():
        ap = tc.alloc_tile_pool(name="v_pool", bufs=1, side="right")
        v_pool["pool"] = ap
        v_pool["w"] = []
        v_pool["x"] = []
        for i in range(IC):
            w = ap.tile([128, VW], BF, name=f"v_w{i}", tag=f"vw{i}", bufs=1)
            nc.sync.dma_start(w[:], io["wvT"][i * 128:(i + 1) * 128, :])
            v_pool["w"].append(w)
            x = ap.tile([128, KTOK], BF, name=f"v_x{i}", tag=f"vx{i}", bufs=1)
            nc.sync.dma_start(x[:], io["vT"][i * 128:(i + 1) * 128, :])
            v_pool["x"].append(x)
        bvb = ap.tile([128, VW], BF, name="v_bb", tag="vbb", bufs=1)
        nc.sync.dma_start(bvb[:], io["bvb"][:])
        v_pool["bb"] = bvb

    def emit_v_tile(part, tt):
        wv_s, vx_s, bvb = v_pool["w"], v_pool["x"], v_pool["bb"]
        c0 = part * 512
        w = 512 if part < 2 else VW - 1024
        ps = psum.tile([128, 512], F32, name="ps_v", tag="proj")
        for i in range(IC):
            nc.tensor.matmul(ps[:, 0:w],
                             vx_s[i][:, tt * 128:(tt + 1) * 128],
                             wv_s[i][:, c0:c0 + w],
                             start=(i == 0), stop=(i == IC - 1))
        nc.vector.scalar_tensor_tensor(
            Vt[tt][:, c0:c0 + w], ps[:, 0:w], 0.0, bvb[:, c0:c0 + w],
            op0=Alu.bypass, op1=Alu.add)

    # ---- attention: row-tiled scores + fp8 exp + PV ----
    ktsl = {}
    E_tiles = {}
    ctx_chunks = {}

    def prefetch_ktsl(p):
        t = attnp.tile([128, KTOK], BF, name=f"ktsl{p}", tag="ktsl", bufs=2)
        nc.sync.dma_start(t[:], kt_store[p][:])
        ktsl[p] = t

    def pair_rounds(p, half2_prev, rest_a, half1_this, rest_b):
        # Slot map (E-pool rotation deadlines, verified against bufs=20):
        #   rounds 0-1: previous pair's PV second halves (4 items/round)
        #   rounds 2-7: rest_a spread
        #   rounds 8-9: this pair's PV first halves (4 items/round)
        #   rounds 10-15: rest_b spread
        kt = ktsl[p]
        EA, EB = [], []
        E_tiles[2 * p] = EA
        E_tiles[2 * p + 1] = EB
        ai = bi = 0
        for r in range(KC):
            ks = slice(r * 128, (r + 1) * 128)
            psA = psum.tile([128, TOK], F32, name="psA", tag="scoresA", bufs=1)
            psB = psum.tile([128, TOK], F32, name="psB", tag="scoresB", bufs=1)
            for qg in range(2):
                qs = slice(qg * 512, (qg + 1) * 512)
                nc.tensor.matmul(psA[:, qs], kt[0:64, ks], Qp[p][0:64, qs],
                                 start=True, stop=True)
                nc.tensor.matmul(psB[:, qs], kt[64:128, ks], Qp[p][64:128, qs],
                                 start=True, stop=True)
            eA = attnp.tile([128, TOK], BF, name="eA", tag="E", bufs=E_BUFS)
            nc.scalar.activation(eA[:], psA[:], Exp,
                                 bias=mask_sb[:, r:r + 1], scale=1.0)
            EA.append(eA)
            eB = attnp.tile([128, TOK], BF, name="eB", tag="E", bufs=E_BUFS)
            nc.scalar.activation(eB[:], psB[:], Exp,
                                 bias=mask_sb[:, r:r + 1], scale=1.0)
            EB.append(eB)
            if r < 2:
                for it in half2_prev[4 * r:4 * r + 4]:
                    it()
            elif r < 8:
                want = len(rest_a) if r == 7 else \
                    (r - 1) * len(rest_a) // 6
                while ai < want:
                    rest_a[ai]()
                    ai += 1
            elif r < 10:
                for it in half1_this[4 * (r - 8):4 * (r - 8) + 4]:
                    it()
            else:
                want = len(rest_b) if r == KC - 1 else \
                    (r - 9) * len(rest_b) // 6
                while bi < want:
                    rest_b[bi]()
                    bi += 1

    def ctx_chunk(p, qt):
        key = (p, qt)
        if key not in ctx_chunks:
            ctx_chunks[key] = attnp.tile([128, 128], BF, name=f"cc{p}_{qt}",
                                         tag="ctxch", bufs=31)
        return ctx_chunks[key]

    partials = {}

    def emit_pv(h, qp, half):
        E = E_tiles[h]
        p, hh = divmod(h, 2)
        vs = slice(h * 65, h * 65 + 65)
        qa, qb = 2 * qp, 2 * qp + 1
        pa = psum.tile([128, HDIM + 1], F32, name="ps_ca", tag="ctx")
        pb = psum.tile([128, HDIM + 1], F32, name="ps_cb", tag="ctx")
        k0, k1 = (0, KC // 2) if half == 0 else (KC // 2, KC)
        for kc in range(k0, k1):
            nc.tensor.matmul(pa[:], E[kc][:, qa * 128:(qa + 1) * 128],
                             Vt[kc][:, vs],
                             start=(kc == k0), stop=(kc == k1 - 1))
            nc.tensor.matmul(pb[:], E[kc][:, qb * 128:(qb + 1) * 128],
                             Vt[kc][:, vs],
                             start=(kc == k0), stop=(kc == k1 - 1))
        for ps, qt in ((pa, qa), (pb, qb)):
            if half == 0:
                pt = attnp.tile([128, HDIM + 1], BF, name="pvp", tag="pvp",
                                bufs=17)
                partials[(h, qt)] = pt
                nc.vector.tensor_copy(pt[:], ps[:])
            else:
                m = attnp.tile([128, HDIM + 1], F32, name="pvm", tag="pvm",
                               bufs=4)
                nc.vector.scalar_tensor_tensor(
                    m[:], ps[:], 0.0, partials[(h, qt)][:],
                    op0=Alu.bypass, op1=Alu.add)
                rec = attnp.tile([128, 1], F32, name="rec", tag="rec", bufs=4)
                nc.vector.reciprocal(rec[:], m[:, 64:65])
                cc = ctx_chunk(p, qt)
                nc.vector.tensor_scalar_mul(cc[:, hh * 64:(hh + 1) * 64],
                                            m[:, 0:64], rec[:])

    # ---- output projection, spread chunk-wise across pairs ----
    o_pool = {}

    def open_o_pool():
        ap = tc.alloc_tile_pool(name="o_pool", bufs=1, side="right")
        o_pool["pool"] = ap
        o_pool["w"] = []
        for i in range(IC):
            w = ap.tile([128, H], BF, name=f"o_w{i}", tag=f"ow{i}", bufs=1)
            nc.sync.dma_start(w[:], io["woT"][i * 128:(i + 1) * 128, :])
            o_pool["w"].append(w)
        bob = ap.tile([128, H], BF, name="o_bb", tag="obb", bufs=1)
        nc.sync.dma_start(bob[:], io["bob"][:])
        o_pool["bb"] = bob
        o_pool["acc"] = [ap.tile([128, H], BF, name=f"oacc{t}", tag=f"oacc{t}",
                                 bufs=1) for t in range(QT)]

    def emit_o_chunk(p, tts):
        ap = o_pool["pool"]
        wo = o_pool["w"][p]
        for tt in tts:
            cc = ctx_chunks[(p, tt)]
            ps_t = psum.tile([128, 128], BF, name="ps_t", tag="ctx")
            nc.tensor.transpose(ps_t[:], cc[:], ident[:])
            tr = ap.tile([128, 128], BF, name="tr", tag="tr", bufs=3)
            nc.vector.tensor_copy(tr[:], ps_t[:])
            pa = psum.tile([128, 512], F32, name="ps_oa", tag="proj")
            pb = psum.tile([128, 512], F32, name="ps_ob", tag="proj")
            last = (p == IC - 1)
            nc.tensor.matmul(pa[:], tr[:], wo[:, 0:512],
                             start=True, stop=True)
            nc.tensor.matmul(pb[:], tr[:], wo[:, 512:1024],
                             start=True, stop=True)
            acc = o_pool["acc"][tt]
            bob = o_pool["bb"]
            if p == 0:
                nc.vector.scalar_tensor_tensor(
                    acc[:, 0:512], pa[:], 0.0, bob[:, 0:512],
                    op0=Alu.bypass, op1=Alu.add)
                nc.vector.scalar_tensor_tensor(
                    acc[:, 512:1024], pb[:], 0.0, bob[:, 512:1024],
                    op0=Alu.bypass, op1=Alu.add)
            elif not last:
                nc.vector.scalar_tensor_tensor(
                    acc[:, 0:512], pa[:], 0.0, acc[:, 0:512],
                    op0=Alu.bypass, op1=Alu.add)
                nc.vector.scalar_tensor_tensor(
                    acc[:, 512:1024], pb[:], 0.0, acc[:, 512:1024],
                    op0=Alu.bypass, op1=Alu.add)
            else:
                osb = ap.tile([128, H], F32, name="osb", tag="osb", bufs=2)
                nc.vector.scalar_tensor_tensor(
                    osb[:, 0:512], pa[:], 0.0, acc[:, 0:512],
                    op0=Alu.bypass, op1=Alu.add)
                nc.vector.scalar_tensor_tensor(
                    osb[:, 512:1024], pb[:], 0.0, acc[:, 512:1024],
                    op0=Alu.bypass, op1=Alu.add)
                nc.sync.dma_start(io["out"][tt * 128:(tt + 1) * 128, :],
                                  osb[:])

    # ---- emission schedule ----
    def pv_items(p, half):
        return [lambda h=h, qp=qp: emit_pv(h, qp, half)
                for h in (2 * p, 2 * p + 1) for qp in range(4)]

    def o_items(c, tts=range(QT)):
        return [lambda c=c, tt=tt: emit_o_chunk(c, (tt,)) for tt in tts]

    def kpf_items(og):
        # K og projection halves + the DRAM->SBUF slab prefetch; placed
        # at the front of the previous pair's filler so the slab is
        # resident well before pair `og` begins.
        def second():
            emit_k_og(og, (1,))
            prefetch_ktsl(og)
        return [lambda og=og: emit_k_og(og, (0,)), second]

    def v_items(part, tts):
        return [lambda part=part, tt=tt: emit_v_tile(part, tt) for tt in tts]

    def swap_qv():
        q_pool["pool"].release()
        open_v_pool()

    def swap_vo():
        v_pool["pool"].release()
        open_o_pool()

    # K inputs first (they gate the first scores via the DRAM slab round
    # trip), K og0 before Q og0 for the same reason.
    open_k_pool()
    open_q_pool()
    emit_k_og(0, (0, 1))
    prefetch_ktsl(0)
    emit_q_og(0)

    # pair 0: Q projection in the early rounds, then part-0 V (all 16
    # tiles must precede this pair's PV first halves at rounds 8-9).
    qog = [lambda og=og: emit_q_og(og) for og in range(1, IC)]
    pair_rounds(0, [],
                kpf_items(1) + qog + [swap_qv] + v_items(0, range(16)),
                pv_items(0, 0),
                [])
    pair_rounds(1, pv_items(0, 1),
                kpf_items(2) + v_items(1, range(8)),
                pv_items(1, 0),
                v_items(1, range(8, 16)) + kpf_items(3))
    pair_rounds(2, pv_items(1, 1),
                kpf_items(4) + v_items(2, range(8)),
                pv_items(2, 0),
                v_items(2, range(8, 16)) + [swap_vo])
    pair_rounds(3, pv_items(2, 1),
                kpf_items(5) + o_items(0),
                pv_items(3, 0),
                o_items(1))
    pair_rounds(4, pv_items(3, 1),
                kpf_items(6) + o_items(2),
                pv_items(4, 0),
                o_items(3))
    pair_rounds(5, pv_items(4, 1),
                kpf_items(7) + o_items(4),
                pv_items(5, 0),
                [lambda: k_pool["pool"].release()])
    pair_rounds(6, pv_items(5, 1), o_items(5), pv_items(6, 0), [])
    pair_rounds(7, pv_items(6, 1), o_items(6), pv_items(7, 0), [])

    # tail: last pair's PV second halves + final O chunk per q-tile pair
    for qp in range(4):
        emit_pv(14, qp, 1)
        emit_pv(15, qp, 1)
        emit_o_chunk(7, (2 * qp, 2 * qp + 1))

    o_pool["pool"].release()
    attnp.release()
    psum.release()
    persist.release()


def _build():
    nc = bacc.Bacc("TRN2", target_bir_lowering=False, debug=False,
                   num_devices=NCORES)
    io = {}

    def inp(name, shape, dtype=BF):
        io[name] = nc.dram_tensor(name, shape, dtype, kind="ExternalInput").ap()
    inp("qT", [H, TOK])
    inp("kT", [H, KTOK])
    inp("vT", [H, KTOK])
    inp("wqT", [H, H])
    inp("wkT", [H, H])
    inp("wvT", [H, VW])
    inp("woT", [H, H])
    inp("bvb", [128, VW])
    inp("bob", [128, H])
    inp("bqcol", [128, IC], F32)
    inp("bkcol", [128, IC], F32)
    inp("maskcol", [128, KC], F32)
    io["out"] = nc.dram_tensor("out", [TOK, H], F32, kind="ExternalOutput").ap()

    with tile.TileContext(nc) as tc:
        _emit(nc, tc, io)
    nc.compile()
    return nc, io


def get_compiled():
    if "nc" not in _CACHE:
        _CACHE["nc"], _CACHE["io"] = _build()
    return _CACHE["nc"]


def make_in_maps(query, key_, value, attention_mask, Wq, bq, Wk, bk, Wv, bv,
                 Wo, bo):
    bf = ml_dtypes.bfloat16
    f32 = np.float32
    query = np.asarray(query, f32)
    key_ = np.asarray(key_, f32)
    value = np.asarray(value, f32)
    attention_mask = np.asarray(attention_mask, f32)
    Wq, bq = np.asarray(Wq, f32), np.asarray(bq, f32)
    Wk, bk = np.asarray(Wk, f32), np.asarray(bk, f32)
    Wv, bv = np.asarray(Wv, f32), np.asarray(bv, f32)
    Wo, bo = np.asarray(Wo, f32), np.asarray(bo, f32)

    scale = 1.0 / np.sqrt(np.float32(HDIM))
    wqT = np.ascontiguousarray((Wq * scale).T).astype(bf)
    wkT = np.ascontiguousarray(Wk.T).astype(bf)
    woT = np.ascontiguousarray(Wo.T).astype(bf)
    wvT = np.zeros((H, VW), f32)
    bv_ext = np.zeros((1, VW), f32)
    for h in range(NH):
        wvT[:, h * 65:h * 65 + 64] = Wv[h * 64:(h + 1) * 64, :].T
        bv_ext[0, h * 65:h * 65 + 64] = bv[h * 64:(h + 1) * 64]
        bv_ext[0, h * 65 + 64] = 1.0
    wvT = wvT.astype(bf)
    bvb = np.broadcast_to(bv_ext, (128, VW)).astype(bf)
    bob = np.broadcast_to(bo.reshape(1, H), (128, H)).astype(bf)
    bqcol = np.ascontiguousarray((bq * scale).reshape(IC, 128).T).astype(f32)
    bkcol = np.ascontiguousarray(bk.reshape(IC, 128).T).astype(f32)

    in_maps = []
    for c in range(NCORES):
        b, half = divmod(c, 2)
        sl = slice(half * TOK, (half + 1) * TOK)
        qT = np.ascontiguousarray(query[b, sl, :].T).astype(bf)
        kT = np.ascontiguousarray(key_[b].T).astype(bf)
        vT = np.ascontiguousarray(value[b].T).astype(bf)
        # -2.0 shift guards fp8e4 overflow in exp(); softmax invariance
        # makes it exact.
        maskcol = np.ascontiguousarray(
            ((1.0 - attention_mask[b]) * -10000.0 - 2.0).reshape(KC, 128).T
        ).astype(f32)
        in_maps.append({
            "qT": qT, "kT": kT, "vT": vT,
            "wqT": wqT, "wkT": wkT, "wvT": wvT, "woT": woT,
            "bvb": bvb, "bob": bob,
            "bqcol": bqcol, "bkcol": bkcol,
            "maskcol": maskcol,
        })
    return in_maps


def kernel(query, key_, value, attention_mask, Wq, bq, Wk, bk, Wv, bv, Wo, bo,
           **run_kwargs):
    nc = get_compiled()
    in_maps = make_in_maps(query, key_, value, attention_mask, Wq, bq, Wk, bk,
                           Wv, bv, Wo, bo)
    res = run_bass_kernel_spmd(nc, in_maps, core_ids=list(range(NCORES)),
                               **run_kwargs)
    out = np.empty((B, S, H), np.float32)
    for c in range(NCORES):
        b, half = divmod(c, 2)
        out[b, half * TOK:(half + 1) * TOK, :] = res.results[c]["out"]
    if run_kwargs:
        kernel.last_results = res
    return out


# revision 20
# speedup vs baseline: 1.0017x; 1.0017x over previous
"""Multi-head attention (B=4, S=2048, H=1024, NH=16) on 8 trn2 NeuronCores.

Sharding: token-parallel, no collectives. Core c handles batch b=c//2,
query half h=c%2 (1024 query tokens), with the full 2048-key K/V of its
batch (K/V projection duplicated within each core pair).

Per-core pipeline v2 (row-tiled scores, fp8 probabilities):
  A) Q projection -> persistent SBUF slabs Qp[og] [128, TOK] bf16 (head
     pair 2og/2og+1 in row halves, no zero padding).  K projection ->
     feature-major slabs spilled to DRAM (og0 kept in SBUF).  V
     projection -> SBUF token-major Vt [tok, 16*65] with per-head ones
     column (PV then also yields the softmax denominator), emitted in
     512/512/16-column parts.  1/8 attention scale folded into Wq.
  B) Attention per head PAIR: scoresT[k,q] for both heads run
     CONCURRENTLY on the PE via K=64 row tiling (rows 0-63 = even head,
     64-127 = odd head) into separate PSUM tiles; exp on ScalarE
     (attention mask + a -2.0 overflow guard as per-partition bias)
     emits fp8e4 E tiles (softmax shift-invariance makes the guard
     free); PV ctx[q, 65] accumulated over 16 key chunks from fp8 E
     (fp8 stationary also gets 4x FWL weight loads), normalized by the
     ones-column denominator into per-pair ctx chunks [128 tok, 128].
  C) Output projection is spread across pairs: as soon as pair p's PV
     completes, its ctx chunk is PE-transposed and matmul'd against
     Wo rows 128p..128p+127, accumulated into SBUF O_partial via DVE;
     the final pair adds the bias and streams the result out, so only
     ~1/8 of the O projection remains after the last exp.
"""

import numpy as np
import ml_dtypes

import concourse.tile as tile
from concourse import bacc, mybir
from concourse.bass_utils import run_bass_kernel_spmd
from concourse.masks import make_identity

B, S, H, NH, HDIM = 4, 2048, 1024, 16, 64
NCORES = 8
TOK = 1024            # query tokens per core
KTOK = 2048           # key tokens per core
IC = H // 128         # 8 feature chunks of 128
KC = KTOK // 128      # 16 key chunks of 128
QT = TOK // 128       # 8 query tiles of 128
NP = IC               # 8 head pairs
VW = NH * (HDIM + 1)  # 1040: V columns incl. per-head ones column
BF = mybir.dt.bfloat16
F32 = mybir.dt.float32
E_BUFS = 20

_CACHE = {}


def _emit(nc, tc, io):
    Exp = mybir.ActivationFunctionType.Exp
    Alu = mybir.AluOpType

    persist = tc.alloc_tile_pool(name="persist", bufs=1)
    psum = tc.alloc_tile_pool(name="psum", bufs=2, space="PSUM")
    attnp = tc.alloc_tile_pool(name="attnp", bufs=1)

    kt_store = nc.dram_tensor("kt_store", [IC, 128, KTOK], BF).ap()

    # ---- persistent tiles ----
    ident = persist.tile([128, 128], BF, name="ident", tag="ident")
    make_identity(nc, ident[:])
    mask_sb = persist.tile([128, KC], F32, name="mask_sb", tag="mask_sb")
    nc.sync.dma_start(mask_sb[:], io["maskcol"][:])
    bqc = persist.tile([128, IC], F32, name="bqc", tag="bqc")
    nc.sync.dma_start(bqc[:], io["bqcol"][:])
    bkc = persist.tile([128, IC], F32, name="bkc", tag="bkc")
    nc.sync.dma_start(bkc[:], io["bkcol"][:])

    Qp = [persist.tile([128, TOK], BF, name=f"Qp{i}", tag=f"Qp{i}")
          for i in range(IC)]
    Vt = [persist.tile([128, VW], BF, name=f"Vt{i}", tag=f"Vt{i}")
          for i in range(KC)]

    # ---- Q projection (og granular) into persistent SBUF slabs ----
    q_pool = {}

    def open_q_pool():
        ap = tc.alloc_tile_pool(name="q_pool", bufs=1, side="right")
        q_pool["pool"] = ap
        q_pool["x"] = []
        q_pool["w"] = []
        for i in range(IC):
            x = ap.tile([128, TOK], BF, name=f"q_x{i}", tag=f"qx{i}")
            nc.sync.dma_start(x[:], io["qT"][i * 128:(i + 1) * 128, :])
            q_pool["x"].append(x)
            w = ap.tile([128, H], BF, name=f"q_w{i}", tag=f"qw{i}")
            nc.sync.dma_start(w[:, 0:512],
                              io["wqT"][i * 128:(i + 1) * 128, 0:512])
            q_pool["w"].append(w)
        for i in range(IC):
            nc.sync.dma_start(q_pool["w"][i][:, 512:1024],
                              io["wqT"][i * 128:(i + 1) * 128, 512:1024])

    def emit_q_og(og):
        x_s, w_s = q_pool["x"], q_pool["w"]
        pa = psum.tile([128, 512], F32, name="ps_qa", tag="proj")
        pb = psum.tile([128, 512], F32, name="ps_qb", tag="proj")
        for i in range(IC):
            w = w_s[i][:, og * 128:(og + 1) * 128]
            nc.tensor.matmul(pa[:], w, x_s[i][:, 0:512],
                             start=(i == 0), stop=(i == IC - 1))
            nc.tensor.matmul(pb[:], w, x_s[i][:, 512:1024],
                             start=(i == 0), stop=(i == IC - 1))
        nc.vector.tensor_scalar_add(Qp[og][:, 0:512], pa[:], bqc[:, og:og + 1])
        nc.vector.tensor_scalar_add(Qp[og][:, 512:1024], pb[:],
                                    bqc[:, og:og + 1])

    # ---- K projection: feature-major slabs to DRAM (og0 in SBUF) ----
    k_pool = {}

    def open_k_pool():
        ap = tc.alloc_tile_pool(name="k_pool", bufs=1)
        k_pool["pool"] = ap
        k_pool["w"] = []
        k_pool["x"] = []
        for i in range(IC):
            w = ap.tile([128, H], BF, name=f"k_w{i}", tag=f"kw{i}", bufs=1)
            nc.sync.dma_start(w[:, 0:512],
                              io["wkT"][i * 128:(i + 1) * 128, 0:512])
            k_pool["w"].append(w)
            x = ap.tile([128, KTOK], BF, name=f"k_x{i}", tag=f"kx{i}", bufs=1)
            nc.sync.dma_start(x[:], io["kT"][i * 128:(i + 1) * 128, :])
            k_pool["x"].append(x)
        for i in range(IC):
            nc.sync.dma_start(k_pool["w"][i][:, 512:1024],
                              io["wkT"][i * 128:(i + 1) * 128, 512:1024])

    def emit_k_og(og, tps):
        ap, w_s, x_s = k_pool["pool"], k_pool["w"], k_pool["x"]
        for tp in tps:
            pa = psum.tile([128, 512], F32, name="ps_ka", tag="proj")
            pb = psum.tile([128, 512], F32, name="ps_kb", tag="proj")
            for i in range(IC):
                w = w_s[i][:, og * 128:(og + 1) * 128]
                nc.tensor.matmul(pa[:], w, x_s[i][:, tp * 1024:tp * 1024 + 512],
                                 start=(i == 0), stop=(i == IC - 1))
                nc.tensor.matmul(pb[:], w,
                                 x_s[i][:, tp * 1024 + 512:tp * 1024 + 1024],
                                 start=(i == 0), stop=(i == IC - 1))
            for ps, tg in ((pa, 0), (pb, 1)):
                col = tp * 1024 + tg * 512
                se = ap.tile([128, 512], BF, name="k_se", tag="kse", bufs=1)
                nc.vector.tensor_scalar_add(se[:], ps[:], bkc[:, og:og + 1])
                nc.sync.dma_start(kt_store[og][:, col:col + 512], se[:])

    # ---- V projection: token-major Vt, 512/512/16 column parts ----
    v_pool = {}

    def open_v_pool():
        ap = tc.alloc_tile_pool(name="v_pool", bufs=1, side="right")
        v_pool["pool"] = ap
        v_pool["w"] = []
        v_pool["x"] = []
        for i in range(IC):
            w = ap.tile([128, VW], BF, name=f"v_w{i}", tag=f"vw{i}", bufs=1)
            nc.sync.dma_start(w[:], io["wvT"][i * 128:(i + 1) * 128, :])
            v_pool["w"].append(w)
            x = ap.tile([128, KTOK], BF, name=f"v_x{i}", tag=f"vx{i}", bufs=1)
            nc.sync.dma_start(x[:], io["vT"][i * 128:(i + 1) * 128, :])
            v_pool["x"].append(x)
        bvb = ap.tile([128, VW], BF, name="v_bb", tag="vbb", bufs=1)
        nc.sync.dma_start(bvb[:], io["bvb"][:])
        v_pool["bb"] = bvb

    def emit_v_tile(part, tt):
        wv_s, vx_s, bvb = v_pool["w"], v_pool["x"], v_pool["bb"]
        c0 = part * 512
        w = 512 if part < 2 else VW - 1024
        ps = psum.tile([128, 512], F32, name="ps_v", tag="proj")
        for i in range(IC):
            nc.tensor.matmul(ps[:, 0:w],
                             vx_s[i][:, tt * 128:(tt + 1) * 128],
                             wv_s[i][:, c0:c0 + w],
                             start=(i == 0), stop=(i == IC - 1))
        nc.vector.scalar_tensor_tensor(
            Vt[tt][:, c0:c0 + w], ps[:, 0:w], 0.0, bvb[:, c0:c0 + w],
            op0=Alu.bypass, op1=Alu.add)

    # ---- attention: row-tiled scores + fp8 exp + PV ----
    ktsl = {}
    E_tiles = {}
    ctx_chunks = {}

    def prefetch_ktsl(p):
        t = attnp.tile([128, KTOK], BF, name=f"ktsl{p}", tag="ktsl", bufs=2)
        nc.sync.dma_start(t[:], kt_store[p][:])
        ktsl[p] = t

    def pair_rounds(p, half2_prev, rest_a, half1_this, rest_b):
        # Slot map (E-pool rotation deadlines, verified against bufs=20):
        #   rounds 0-1: previous pair's PV second halves (4 items/round)
        #   rounds 2-7: rest_a spread
        #   rounds 8-9: this pair's PV first halves (4 items/round)
        #   rounds 10-15: rest_b spread
        kt = ktsl[p]
        EA, EB = [], []
        E_tiles[2 * p] = EA
        E_tiles[2 * p + 1] = EB
        ai = bi = 0
        for r in range(KC):
            ks = slice(r * 128, (r + 1) * 128)
            psA = psum.tile([128, TOK], F32, name="psA", tag="scoresA", bufs=1)
            psB = psum.tile([128, TOK], F32, name="psB", tag="scoresB", bufs=1)
            for qg in range(2):
                qs = slice(qg * 512, (qg + 1) * 512)
                nc.tensor.matmul(psA[:, qs], kt[0:64, ks], Qp[p][0:64, qs],
                                 start=True, stop=True)
                nc.tensor.matmul(psB[:, qs], kt[64:128, ks], Qp[p][64:128, qs],
                                 start=True, stop=True)
            eA = attnp.tile([128, TOK], BF, name="eA", tag="E", bufs=E_BUFS)
            nc.scalar.activation(eA[:], psA[:], Exp,
                                 bias=mask_sb[:, r:r + 1], scale=1.0)
            EA.append(eA)
            eB = attnp.tile([128, TOK], BF, name="eB", tag="E", bufs=E_BUFS)
            nc.scalar.activation(eB[:], psB[:], Exp,
                                 bias=mask_sb[:, r:r + 1], scale=1.0)
            EB.append(eB)
            if r < 2:
                for it in half2_prev[4 * r:4 * r + 4]:
                    it()
            elif r < 8:
                want = len(rest_a) if r == 7 else \
                    (r - 1) * len(rest_a) // 6
                while ai < want:
                    rest_a[ai]()
                    ai += 1
            elif r < 10:
                for it in half1_this[4 * (r - 8):4 * (r - 8) + 4]:
                    it()
            else:
                want = len(rest_b) if r == KC - 1 else \
                    (r - 9) * len(rest_b) // 6
                while bi < want:
                    rest_b[bi]()
                    bi += 1

    def ctx_chunk(p, qt):
        key = (p, qt)
        if key not in ctx_chunks:
            ctx_chunks[key] = attnp.tile([128, 128], BF, name=f"cc{p}_{qt}",
                                         tag="ctxch", bufs=31)
        return ctx_chunks[key]

    partials = {}

    def emit_pv(h, qp, half):
        E = E_tiles[h]
        p, hh = divmod(h, 2)
        vs = slice(h * 65, h * 65 + 65)
        qa, qb = 2 * qp, 2 * qp + 1
        pa = psum.tile([128, HDIM + 1], F32, name="ps_ca", tag="ctx")
        pb = psum.tile([128, HDIM + 1], F32, name="ps_cb", tag="ctx")
        k0, k1 = (0, KC // 2) if half == 0 else (KC // 2, KC)
        for kc in range(k0, k1):
            nc.tensor.matmul(pa[:], E[kc][:, qa * 128:(qa + 1) * 128],
                             Vt[kc][:, vs],
                             start=(kc == k0), stop=(kc == k1 - 1))
            nc.tensor.matmul(pb[:], E[kc][:, qb * 128:(qb + 1) * 128],
                             Vt[kc][:, vs],
                             start=(kc == k0), stop=(kc == k1 - 1))
        for ps, qt in ((pa, qa), (pb, qb)):
            if half == 0:
                pt = attnp.tile([128, HDIM + 1], BF, name="pvp", tag="pvp",
                                bufs=17)
                partials[(h, qt)] = pt
                nc.vector.tensor_copy(pt[:], ps[:])
            else:
                m = attnp.tile([128, HDIM + 1], F32, name="pvm", tag="pvm",
                               bufs=4)
                nc.vector.scalar_tensor_tensor(
                    m[:], ps[:], 0.0, partials[(h, qt)][:],
                    op0=Alu.bypass, op1=Alu.add)
                rec = attnp.tile([128, 1], F32, name="rec", tag="rec", bufs=4)
                nc.vector.reciprocal(rec[:], m[:, 64:65])
                cc = ctx_chunk(p, qt)
                nc.vector.tensor_scalar_mul(cc[:, hh * 64:(hh + 1) * 64],
                                            m[:, 0:64], rec[:])

    # ---- output projection, spread chunk-wise across pairs ----
    o_pool = {}

    def open_o_pool():
        ap = tc.alloc_tile_pool(name="o_pool", bufs=1, side="right")
        o_pool["pool"] = ap
        o_pool["w"] = []
        for i in range(IC):
            w = ap.tile([128, H], BF, name=f"o_w{i}", tag=f"ow{i}", bufs=1)
            nc.sync.dma_start(w[:], io["woT"][i * 128:(i + 1) * 128, :])
            o_pool["w"].append(w)
        bob = ap.tile([128, H], BF, name="o_bb", tag="obb", bufs=1)
        nc.sync.dma_start(bob[:], io["bob"][:])
        o_pool["bb"] = bob
        o_pool["acc"] = [ap.tile([128, H], BF, name=f"oacc{t}", tag=f"oacc{t}",
                                 bufs=1) for t in range(QT)]

    def emit_o_chunk(p, tts):
        ap = o_pool["pool"]
        wo = o_pool["w"][p]
        for tt in tts:
            cc = ctx_chunks[(p, tt)]
            ps_t = psum.tile([128, 128], BF, name="ps_t", tag="ctx")
            nc.tensor.transpose(ps_t[:], cc[:], ident[:])
            tr = ap.tile([128, 128], BF, name="tr", tag="tr", bufs=3)
            nc.vector.tensor_copy(tr[:], ps_t[:])
            pa = psum.tile([128, 512], F32, name="ps_oa", tag="proj")
            pb = psum.tile([128, 512], F32, name="ps_ob", tag="proj")
            last = (p == IC - 1)
            nc.tensor.matmul(pa[:], tr[:], wo[:, 0:512],
                             start=True, stop=True)
            nc.tensor.matmul(pb[:], tr[:], wo[:, 512:1024],
                             start=True, stop=True)
            acc = o_pool["acc"][tt]
            bob = o_pool["bb"]
            if p == 0:
                nc.vector.scalar_tensor_tensor(
                    acc[:, 0:512], pa[:], 0.0, bob[:, 0:512],
                    op0=Alu.bypass, op1=Alu.add)
                nc.vector.scalar_tensor_tensor(
                    acc[:, 512:1024], pb[:], 0.0, bob[:, 512:1024],
                    op0=Alu.bypass, op1=Alu.add)
            elif not last:
                nc.vector.scalar_tensor_tensor(
                    acc[:, 0:512], pa[:], 0.0, acc[:, 0:512],
                    op0=Alu.bypass, op1=Alu.add)
                nc.vector.scalar_tensor_tensor(
                    acc[:, 512:1024], pb[:], 0.0, acc[:, 512:1024],
                    op0=Alu.bypass, op1=Alu.add)
            else:
                osb = ap.tile([128, H], F32, name="osb", tag="osb", bufs=2)
                nc.vector.scalar_tensor_tensor(
                    osb[:, 0:512], pa[:], 0.0, acc[:, 0:512],
                    op0=Alu.bypass, op1=Alu.add)
                nc.vector.scalar_tensor_tensor(
                    osb[:, 512:1024], pb[:], 0.0, acc[:, 512:1024],
                    op0=Alu.bypass, op1=Alu.add)
                nc.sync.dma_start(io["out"][tt * 128:(tt + 1) * 128, :],
                                  osb[:])

    # ---- emission schedule ----
    def pv_items(p, half):
        return [lambda h=h, qp=qp: emit_pv(h, qp, half)
                for h in (2 * p, 2 * p + 1) for qp in range(4)]

    def o_items(c, tts=range(QT)):
        return [lambda c=c, tt=tt: emit_o_chunk(c, (tt,)) for tt in tts]

    def kpf_items(og):
        # K og projection halves + the DRAM->SBUF slab prefetch; placed
        # at the front of the previous pair's filler so the slab is
        # resident well before pair `og` begins.
        def second():
            emit_k_og(og, (1,))
            prefetch_ktsl(og)
        return [lambda og=og: emit_k_og(og, (0,)), second]

    def v_items(part, tts):
        return [lambda part=part, tt=tt: emit_v_tile(part, tt) for tt in tts]

    def swap_qv():
        q_pool["pool"].release()
        open_v_pool()

    def swap_vo():
        v_pool["pool"].release()
        open_o_pool()

    # K inputs first (they gate the first scores via the DRAM slab round
    # trip), K og0 before Q og0 for the same reason.
    open_k_pool()
    open_q_pool()
    emit_k_og(0, (0, 1))
    prefetch_ktsl(0)

    # Upfront: full Q projection and part-0 V, so the exp stream runs
    # without the large pair-0/1 boundary stall once it starts.
    for og in range(IC):
        emit_q_og(og)
    swap_qv()
    for tt in range(KC):
        emit_v_tile(0, tt)

    pair_rounds(0, [], kpf_items(1), pv_items(0, 0), [])
    pair_rounds(1, pv_items(0, 1),
                kpf_items(2) + v_items(1, range(8)),
                pv_items(1, 0),
                v_items(1, range(8, 16)) + kpf_items(3))
    pair_rounds(2, pv_items(1, 1),
                kpf_items(4) + v_items(2, range(8)),
                pv_items(2, 0),
                v_items(2, range(8, 16)) + [swap_vo])
    pair_rounds(3, pv_items(2, 1),
                kpf_items(5) + o_items(0),
                pv_items(3, 0),
                o_items(1))
    pair_rounds(4, pv_items(3, 1),
                kpf_items(6) + o_items(2),
                pv_items(4, 0),
                o_items(3))
    pair_rounds(5, pv_items(4, 1),
                kpf_items(7) + o_items(4),
                pv_items(5, 0),
                [lambda: k_pool["pool"].release()])
    pair_rounds(6, pv_items(5, 1), o_items(5), pv_items(6, 0), [])
    pair_rounds(7, pv_items(6, 1), o_items(6), pv_items(7, 0), [])

    # tail: last pair's PV second halves + final O chunk per q-tile pair
    for qp in range(4):
        emit_pv(14, qp, 1)
        emit_pv(15, qp, 1)
        emit_o_chunk(7, (2 * qp, 2 * qp + 1))

    o_pool["pool"].release()
    attnp.release()
    psum.release()
    persist.release()


def _build():
    nc = bacc.Bacc("TRN2", target_bir_lowering=False, debug=False,
                   num_devices=NCORES)
    io = {}

    def inp(name, shape, dtype=BF):
        io[name] = nc.dram_tensor(name, shape, dtype, kind="ExternalInput").ap()
    inp("qT", [H, TOK])
    inp("kT", [H, KTOK])
    inp("vT", [H, KTOK])
    inp("wqT", [H, H])
    inp("wkT", [H, H])
    inp("wvT", [H, VW])
    inp("woT", [H, H])
    inp("bvb", [128, VW])
    inp("bob", [128, H])
    inp("bqcol", [128, IC], F32)
    inp("bkcol", [128, IC], F32)
    inp("maskcol", [128, KC], F32)
    io["out"] = nc.dram_tensor("out", [TOK, H], F32, kind="ExternalOutput").ap()

    with tile.TileContext(nc) as tc:
        _emit(nc, tc, io)
    nc.compile()
    return nc, io


def get_compiled():
    if "nc" not in _CACHE:
        _CACHE["nc"], _CACHE["io"] = _build()
    return _CACHE["nc"]


def make_in_maps(query, key_, value, attention_mask, Wq, bq, Wk, bk, Wv, bv,
                 Wo, bo):
    bf = ml_dtypes.bfloat16
    f32 = np.float32
    query = np.asarray(query, f32)
    key_ = np.asarray(key_, f32)
    value = np.asarray(value, f32)
    attention_mask = np.asarray(attention_mask, f32)
    Wq, bq = np.asarray(Wq, f32), np.asarray(bq, f32)
    Wk, bk = np.asarray(Wk, f32), np.asarray(bk, f32)
    Wv, bv = np.asarray(Wv, f32), np.asarray(bv, f32)
    Wo, bo = np.asarray(Wo, f32), np.asarray(bo, f32)

    scale = 1.0 / np.sqrt(np.float32(HDIM))
    wqT = np.ascontiguousarray((Wq * scale).T).astype(bf)
    wkT = np.ascontiguousarray(Wk.T).astype(bf)
    woT = np.ascontiguousarray(Wo.T).astype(bf)
    wvT = np.zeros((H, VW), f32)
    bv_ext = np.zeros((1, VW), f32)
    for h in range(NH):
        wvT[:, h * 65:h * 65 + 64] = Wv[h * 64:(h + 1) * 64, :].T
        bv_ext[0, h * 65:h * 65 + 64] = bv[h * 64:(h + 1) * 64]
        bv_ext[0, h * 65 + 64] = 1.0
    wvT = wvT.astype(bf)
    bvb = np.broadcast_to(bv_ext, (128, VW)).astype(bf)
    bob = np.broadcast_to(bo.reshape(1, H), (128, H)).astype(bf)
    bqcol = np.ascontiguousarray((bq * scale).reshape(IC, 128).T).astype(f32)
    bkcol = np.ascontiguousarray(bk.reshape(IC, 128).T).astype(f32)

    in_maps = []
    for c in range(NCORES):
        b, half = divmod(c, 2)
        sl = slice(half * TOK, (half + 1) * TOK)
        qT = np.ascontiguousarray(query[b, sl, :].T).astype(bf)
        kT = np.ascontiguousarray(key_[b].T).astype(bf)
        vT = np.ascontiguousarray(value[b].T).astype(bf)
        # -2.0 shift guards fp8e4 overflow in exp(); softmax invariance
        # makes it exact.
        maskcol = np.ascontiguousarray(
            ((1.0 - attention_mask[b]) * -10000.0 - 2.0).reshape(KC, 128).T
        ).astype(f32)
        in_maps.append({
            "qT": qT, "kT": kT, "vT": vT,
            "wqT": wqT, "wkT": wkT, "wvT": wvT, "woT": woT,
            "bvb": bvb, "bob": bob,
            "bqcol": bqcol, "bkcol": bkcol,
            "maskcol": maskcol,
        })
    return in_maps


def kernel(query, key_, value, attention_mask, Wq, bq, Wk, bk, Wv, bv, Wo, bo,
           **run_kwargs):
    nc = get_compiled()
    in_maps = make_in_maps(query, key_, value, attention_mask, Wq, bq, Wk, bk,
                           Wv, bv, Wo, bo)
    res = run_bass_kernel_spmd(nc, in_maps, core_ids=list(range(NCORES)),
                               **run_kwargs)
    out = np.empty((B, S, H), np.float32)
    for c in range(NCORES):
        b, half = divmod(c, 2)
        out[b, half * TOK:(half + 1) * TOK, :] = res.results[c]["out"]
    if run_kwargs:
        kernel.last_results = res
    return out


# revision 23
# speedup vs baseline: 1.0891x; 1.0872x over previous
"""Multi-head attention (B=4, S=2048, H=1024, NH=16) on 8 trn2 NeuronCores.

Sharding: token-parallel, no collectives. Core c handles batch b=c//2,
query half h=c%2 (1024 query tokens), with the full 2048-key K/V of its
batch (K/V projection duplicated within each core pair).

Per-core pipeline v3 (row-tiled scores, split PV):
  A) Q projection -> persistent SBUF slabs Qp[og] [128, TOK] bf16 (head
     pair 2og/2og+1 in row halves, no zero padding).  K projection ->
     feature-major slabs spilled to DRAM, reloaded per pair.  V
     projection -> SBUF token-major Vt [tok, 16*65] with per-head ones
     column (PV then also yields the softmax denominator), emitted in
     512/512/16-column parts; V/O biases folded in via DVE broadcast
     adds (no K=1 bias matmuls).  1/8 attention scale folded into Wq.
  B) Attention per head PAIR: scoresT[k,q] for both heads run
     CONCURRENTLY on the PE via K=64 row tiling (rows 0-63 = even head,
     64-127 = odd head) into separate PSUM tiles; exp on ScalarE emits
     bf16 E tiles.  PV is split into key-halves so the E pool needs
     only 20 bufs: kc 0-7 accumulated mid-pair into SBUF partials,
     kc 8-15 at the next pair's start, merged + normalized by the
     ones-column denominator into per-pair ctx chunks [128 tok, 128].
  C) Output projection is spread across pairs: as soon as pair p's PV
     completes, its ctx chunk is PE-transposed and matmul'd against
     Wo rows 128p..128p+127, accumulated into SBUF O_partial via DVE;
     the final pair adds the bias and streams the result out, so only
     ~1/8 of the O projection remains after the last exp.
"""

import numpy as np
import ml_dtypes

import concourse.tile as tile
from concourse import bacc, mybir
from concourse.bass_utils import run_bass_kernel_spmd
from concourse.masks import make_identity

B, S, H, NH, HDIM = 4, 2048, 1024, 16, 64
NCORES = 8
TOK = 1024            # query tokens per core
KTOK = 2048           # key tokens per core
IC = H // 128         # 8 feature chunks of 128
KC = KTOK // 128      # 16 key chunks of 128
QT = TOK // 128       # 8 query tiles of 128
NP = IC               # 8 head pairs
VW = NH * (HDIM + 1)  # 1040: V columns incl. per-head ones column
BF = mybir.dt.bfloat16
F32 = mybir.dt.float32
E_BUFS = 20

_CACHE = {}


def _emit(nc, tc, io):
    Exp = mybir.ActivationFunctionType.Exp
    Alu = mybir.AluOpType

    persist = tc.alloc_tile_pool(name="persist", bufs=1)
    psum = tc.alloc_tile_pool(name="psum", bufs=2, space="PSUM")
    attnp = tc.alloc_tile_pool(name="attnp", bufs=1)

    kt_store = nc.dram_tensor("kt_store", [IC, 128, KTOK], BF).ap()

    # ---- persistent tiles ----
    ident = persist.tile([128, 128], BF, name="ident", tag="ident")
    make_identity(nc, ident[:])
    mask_sb = persist.tile([128, KC], F32, name="mask_sb", tag="mask_sb")
    nc.sync.dma_start(mask_sb[:], io["maskcol"][:])
    bqc = persist.tile([128, IC], F32, name="bqc", tag="bqc")
    nc.sync.dma_start(bqc[:], io["bqcol"][:])
    bkc = persist.tile([128, IC], F32, name="bkc", tag="bkc")
    nc.sync.dma_start(bkc[:], io["bkcol"][:])

    Qp = [persist.tile([128, TOK], BF, name=f"Qp{i}", tag=f"Qp{i}")
          for i in range(IC)]
    Vt = [persist.tile([128, VW], BF, name=f"Vt{i}", tag=f"Vt{i}")
          for i in range(KC)]

    # ---- Q projection (og granular) into persistent SBUF slabs ----
    q_pool = {}

    def open_q_pool():
        ap = tc.alloc_tile_pool(name="q_pool", bufs=1, side="right")
        q_pool["pool"] = ap
        q_pool["x"] = []
        q_pool["w"] = []
        for i in range(IC):
            x = ap.tile([128, TOK], BF, name=f"q_x{i}", tag=f"qx{i}")
            nc.sync.dma_start(x[:], io["qT"][i * 128:(i + 1) * 128, :])
            q_pool["x"].append(x)
            w = ap.tile([128, H], BF, name=f"q_w{i}", tag=f"qw{i}")
            nc.sync.dma_start(w[:, 0:512],
                              io["wqT"][i * 128:(i + 1) * 128, 0:512])
            q_pool["w"].append(w)
        for i in range(IC):
            nc.sync.dma_start(q_pool["w"][i][:, 512:1024],
                              io["wqT"][i * 128:(i + 1) * 128, 512:1024])

    def emit_q_og(og):
        x_s, w_s = q_pool["x"], q_pool["w"]
        pa = psum.tile([128, 512], F32, name="ps_qa", tag="proj")
        pb = psum.tile([128, 512], F32, name="ps_qb", tag="proj")
        for i in range(IC):
            w = w_s[i][:, og * 128:(og + 1) * 128]
            nc.tensor.matmul(pa[:], w, x_s[i][:, 0:512],
                             start=(i == 0), stop=(i == IC - 1))
            nc.tensor.matmul(pb[:], w, x_s[i][:, 512:1024],
                             start=(i == 0), stop=(i == IC - 1))
        nc.vector.tensor_scalar_add(Qp[og][:, 0:512], pa[:], bqc[:, og:og + 1])
        nc.vector.tensor_scalar_add(Qp[og][:, 512:1024], pb[:],
                                    bqc[:, og:og + 1])

    # ---- K projection: feature-major slabs to DRAM (og0 in SBUF) ----
    k_pool = {}

    def open_k_pool():
        ap = tc.alloc_tile_pool(name="k_pool", bufs=1)
        k_pool["pool"] = ap
        k_pool["w"] = []
        k_pool["x"] = []
        for i in range(IC):
            w = ap.tile([128, H], BF, name=f"k_w{i}", tag=f"kw{i}", bufs=1)
            nc.sync.dma_start(w[:, 0:512],
                              io["wkT"][i * 128:(i + 1) * 128, 0:512])
            k_pool["w"].append(w)
            x = ap.tile([128, KTOK], BF, name=f"k_x{i}", tag=f"kx{i}", bufs=1)
            nc.sync.dma_start(x[:], io["kT"][i * 128:(i + 1) * 128, :])
            k_pool["x"].append(x)
        for i in range(IC):
            nc.sync.dma_start(k_pool["w"][i][:, 512:1024],
                              io["wkT"][i * 128:(i + 1) * 128, 512:1024])

    def emit_k_og(og, tps):
        ap, w_s, x_s = k_pool["pool"], k_pool["w"], k_pool["x"]
        for tp in tps:
            pa = psum.tile([128, 512], F32, name="ps_ka", tag="proj")
            pb = psum.tile([128, 512], F32, name="ps_kb", tag="proj")
            for i in range(IC):
                w = w_s[i][:, og * 128:(og + 1) * 128]
                nc.tensor.matmul(pa[:], w, x_s[i][:, tp * 1024:tp * 1024 + 512],
                                 start=(i == 0), stop=(i == IC - 1))
                nc.tensor.matmul(pb[:], w,
                                 x_s[i][:, tp * 1024 + 512:tp * 1024 + 1024],
                                 start=(i == 0), stop=(i == IC - 1))
            for ps, tg in ((pa, 0), (pb, 1)):
                col = tp * 1024 + tg * 512
                se = ap.tile([128, 512], BF, name="k_se", tag="kse", bufs=1)
                nc.vector.tensor_scalar_add(se[:], ps[:], bkc[:, og:og + 1])
                nc.sync.dma_start(kt_store[og][:, col:col + 512], se[:])

    # ---- V projection: token-major Vt, 512/512/16 column parts ----
    v_pool = {}

    def open_v_pool():
        ap = tc.alloc_tile_pool(name="v_pool", bufs=1, side="right")
        v_pool["pool"] = ap
        v_pool["w"] = []
        v_pool["x"] = []
        for i in range(IC):
            w = ap.tile([128, VW], BF, name=f"v_w{i}", tag=f"vw{i}", bufs=1)
            nc.sync.dma_start(w[:], io["wvT"][i * 128:(i + 1) * 128, :])
            v_pool["w"].append(w)
            x = ap.tile([128, KTOK], BF, name=f"v_x{i}", tag=f"vx{i}", bufs=1)
            nc.sync.dma_start(x[:], io["vT"][i * 128:(i + 1) * 128, :])
            v_pool["x"].append(x)
        bvb = ap.tile([128, VW], BF, name="v_bb", tag="vbb", bufs=1)
        nc.sync.dma_start(bvb[:], io["bvb"][:])
        v_pool["bb"] = bvb

    def emit_v_tile(part, tt):
        wv_s, vx_s, bvb = v_pool["w"], v_pool["x"], v_pool["bb"]
        c0 = part * 512
        w = 512 if part < 2 else VW - 1024
        ps = psum.tile([128, 512], F32, name="ps_v", tag="proj")
        for i in range(IC):
            nc.tensor.matmul(ps[:, 0:w],
                             vx_s[i][:, tt * 128:(tt + 1) * 128],
                             wv_s[i][:, c0:c0 + w],
                             start=(i == 0), stop=(i == IC - 1))
        nc.vector.scalar_tensor_tensor(
            Vt[tt][:, c0:c0 + w], ps[:, 0:w], 0.0, bvb[:, c0:c0 + w],
            op0=Alu.bypass, op1=Alu.add)

    # ---- attention: row-tiled scores + fp8 exp + PV ----
    ktsl = {}
    E_tiles = {}
    ctx_chunks = {}

    def prefetch_ktsl(p):
        t = attnp.tile([128, KTOK], BF, name=f"ktsl{p}", tag="ktsl", bufs=2)
        nc.sync.dma_start(t[:], kt_store[p][:])
        ktsl[p] = t

    def pair_rounds(p, half2_prev, rest_a, half1_this, rest_b):
        # Slot map (E-pool rotation deadlines, verified against bufs=20):
        #   rounds 0-1: previous pair's PV second halves (4 items/round)
        #   rounds 2-7: rest_a spread
        #   rounds 8-9: this pair's PV first halves (4 items/round)
        #   rounds 10-15: rest_b spread
        kt = ktsl[p]
        EA, EB = [], []
        E_tiles[2 * p] = EA
        E_tiles[2 * p + 1] = EB
        ai = bi = 0
        for r in range(KC):
            ks = slice(r * 128, (r + 1) * 128)
            psA = psum.tile([128, TOK], F32, name="psA", tag="scoresA", bufs=1)
            psB = psum.tile([128, TOK], F32, name="psB", tag="scoresB", bufs=1)
            for qg in range(2):
                qs = slice(qg * 512, (qg + 1) * 512)
                nc.tensor.matmul(psA[:, qs], kt[0:64, ks], Qp[p][0:64, qs],
                                 start=True, stop=True)
                nc.tensor.matmul(psB[:, qs], kt[64:128, ks], Qp[p][64:128, qs],
                                 start=True, stop=True)
            eA = attnp.tile([128, TOK], BF, name="eA", tag="E", bufs=E_BUFS)
            nc.scalar.activation(eA[:], psA[:], Exp,
                                 bias=mask_sb[:, r:r + 1], scale=1.0)
            EA.append(eA)
            eB = attnp.tile([128, TOK], BF, name="eB", tag="E", bufs=E_BUFS)
            nc.scalar.activation(eB[:], psB[:], Exp,
                                 bias=mask_sb[:, r:r + 1], scale=1.0)
            EB.append(eB)
            if r < 2:
                for it in half2_prev[4 * r:4 * r + 4]:
                    it()
            elif r < 8:
                want = len(rest_a) if r == 7 else \
                    (r - 1) * len(rest_a) // 6
                while ai < want:
                    rest_a[ai]()
                    ai += 1
            elif r < 10:
                for it in half1_this[4 * (r - 8):4 * (r - 8) + 4]:
                    it()
            else:
                want = len(rest_b) if r == KC - 1 else \
                    (r - 9) * len(rest_b) // 6
                while bi < want:
                    rest_b[bi]()
                    bi += 1

    def ctx_chunk(p, qt):
        key = (p, qt)
        if key not in ctx_chunks:
            ctx_chunks[key] = attnp.tile([128, 128], BF, name=f"cc{p}_{qt}",
                                         tag="ctxch", bufs=31)
        return ctx_chunks[key]

    partials = {}

    def emit_pv(h, qp, half):
        E = E_tiles[h]
        p, hh = divmod(h, 2)
        vs = slice(h * 65, h * 65 + 65)
        qa, qb = 2 * qp, 2 * qp + 1
        pa = psum.tile([128, HDIM + 1], F32, name="ps_ca", tag="ctx")
        pb = psum.tile([128, HDIM + 1], F32, name="ps_cb", tag="ctx")
        k0, k1 = (0, KC // 2) if half == 0 else (KC // 2, KC)
        for kc in range(k0, k1):
            nc.tensor.matmul(pa[:], E[kc][:, qa * 128:(qa + 1) * 128],
                             Vt[kc][:, vs],
                             start=(kc == k0), stop=(kc == k1 - 1))
            nc.tensor.matmul(pb[:], E[kc][:, qb * 128:(qb + 1) * 128],
                             Vt[kc][:, vs],
                             start=(kc == k0), stop=(kc == k1 - 1))
        for ps, qt in ((pa, qa), (pb, qb)):
            if half == 0:
                pt = attnp.tile([128, HDIM + 1], BF, name="pvp", tag="pvp",
                                bufs=17)
                partials[(h, qt)] = pt
                nc.vector.tensor_copy(pt[:], ps[:])
            else:
                m = attnp.tile([128, HDIM + 1], F32, name="pvm", tag="pvm",
                               bufs=4)
                nc.vector.scalar_tensor_tensor(
                    m[:], ps[:], 0.0, partials[(h, qt)][:],
                    op0=Alu.bypass, op1=Alu.add)
                rec = attnp.tile([128, 1], F32, name="rec", tag="rec", bufs=4)
                nc.vector.reciprocal(rec[:], m[:, 64:65])
                cc = ctx_chunk(p, qt)
                nc.vector.tensor_scalar_mul(cc[:, hh * 64:(hh + 1) * 64],
                                            m[:, 0:64], rec[:])

    # ---- output projection, spread chunk-wise across pairs ----
    o_pool = {}

    def open_o_pool():
        ap = tc.alloc_tile_pool(name="o_pool", bufs=1, side="right")
        o_pool["pool"] = ap
        o_pool["w"] = []
        for i in range(IC):
            w = ap.tile([128, H], BF, name=f"o_w{i}", tag=f"ow{i}", bufs=1)
            nc.sync.dma_start(w[:], io["woT"][i * 128:(i + 1) * 128, :])
            o_pool["w"].append(w)
        bob = ap.tile([128, H], BF, name="o_bb", tag="obb", bufs=1)
        nc.sync.dma_start(bob[:], io["bob"][:])
        o_pool["bb"] = bob
        o_pool["acc"] = [ap.tile([128, H], BF, name=f"oacc{t}", tag=f"oacc{t}",
                                 bufs=1) for t in range(QT)]

    def emit_o_chunk(p, tts):
        ap = o_pool["pool"]
        wo = o_pool["w"][p]
        for tt in tts:
            cc = ctx_chunks[(p, tt)]
            ps_t = psum.tile([128, 128], BF, name="ps_t", tag="ctx")
            nc.tensor.transpose(ps_t[:], cc[:], ident[:])
            tr = ap.tile([128, 128], BF, name="tr", tag="tr", bufs=3)
            nc.vector.tensor_copy(tr[:], ps_t[:])
            pa = psum.tile([128, 512], F32, name="ps_oa", tag="proj")
            pb = psum.tile([128, 512], F32, name="ps_ob", tag="proj")
            last = (p == IC - 1)
            nc.tensor.matmul(pa[:], tr[:], wo[:, 0:512],
                             start=True, stop=True)
            nc.tensor.matmul(pb[:], tr[:], wo[:, 512:1024],
                             start=True, stop=True)
            acc = o_pool["acc"][tt]
            bob = o_pool["bb"]
            if p == 0:
                nc.vector.scalar_tensor_tensor(
                    acc[:, 0:512], pa[:], 0.0, bob[:, 0:512],
                    op0=Alu.bypass, op1=Alu.add)
                nc.vector.scalar_tensor_tensor(
                    acc[:, 512:1024], pb[:], 0.0, bob[:, 512:1024],
                    op0=Alu.bypass, op1=Alu.add)
            elif not last:
                nc.vector.scalar_tensor_tensor(
                    acc[:, 0:512], pa[:], 0.0, acc[:, 0:512],
                    op0=Alu.bypass, op1=Alu.add)
                nc.vector.scalar_tensor_tensor(
                    acc[:, 512:1024], pb[:], 0.0, acc[:, 512:1024],
                    op0=Alu.bypass, op1=Alu.add)
            else:
                osb = ap.tile([128, H], F32, name="osb", tag="osb", bufs=2)
                nc.vector.scalar_tensor_tensor(
                    osb[:, 0:512], pa[:], 0.0, acc[:, 0:512],
                    op0=Alu.bypass, op1=Alu.add)
                nc.vector.scalar_tensor_tensor(
                    osb[:, 512:1024], pb[:], 0.0, acc[:, 512:1024],
                    op0=Alu.bypass, op1=Alu.add)
                nc.sync.dma_start(io["out"][tt * 128:(tt + 1) * 128, :],
                                  osb[:])

    # ---- emission schedule ----
    def pv_items(p, half):
        return [lambda h=h, qp=qp: emit_pv(h, qp, half)
                for h in (2 * p, 2 * p + 1) for qp in range(4)]

    def o_items(c):
        return [lambda c=c, t=4 * j: emit_o_chunk(c, (t, t + 1, t + 2, t + 3))
                for j in range(2)]

    def k_items(og):
        return [lambda og=og, tp=tp: emit_k_og(og, (tp,)) for tp in (0, 1)]

    def v_items(part, tts):
        return [lambda part=part, tt=tt: emit_v_tile(part, tt) for tt in tts]

    def swap_qv():
        q_pool["pool"].release()
        open_v_pool()

    def swap_vo():
        v_pool["pool"].release()
        open_o_pool()

    # K inputs first (they gate the first scores via the DRAM slab round
    # trip), K og0 before Q og0 for the same reason.
    open_q_pool()
    emit_q_og(0)
    open_k_pool()
    emit_k_og(0, (0, 1))
    prefetch_ktsl(0)

    # pair 0: Q projection in the early rounds, then part-0 V (all 16
    # tiles must precede this pair's PV first halves at rounds 8-9).
    qog = [lambda og=og: emit_q_og(og) for og in range(1, IC)]
    pair_rounds(0, [],
                qog + [swap_qv] + v_items(0, range(16)),
                pv_items(0, 0),
                k_items(1))
    prefetch_ktsl(1)

    pair_rounds(1, pv_items(0, 1),
                v_items(1, range(8)) + k_items(2),
                pv_items(1, 0),
                v_items(1, range(8, 16)) + k_items(3))
    prefetch_ktsl(2)
    pair_rounds(2, pv_items(1, 1),
                v_items(2, range(8)) + k_items(4),
                pv_items(2, 0),
                v_items(2, range(8, 16)) + k_items(5))
    prefetch_ktsl(3)
    pair_rounds(3, pv_items(2, 1),
                k_items(6),
                pv_items(3, 0),
                [swap_vo] + k_items(7))
    prefetch_ktsl(4)
    pair_rounds(4, pv_items(3, 1),
                o_items(0) + [lambda: k_pool["pool"].release()],
                pv_items(4, 0),
                o_items(1))
    prefetch_ktsl(5)
    pair_rounds(5, pv_items(4, 1), o_items(2), pv_items(5, 0), o_items(3))
    prefetch_ktsl(6)
    pair_rounds(6, pv_items(5, 1), o_items(4), pv_items(6, 0), o_items(5))
    prefetch_ktsl(7)
    pair_rounds(7, pv_items(6, 1), o_items(6), pv_items(7, 0), [])

    # tail: last pair's PV second halves + final O chunk per q-tile pair
    for qp in range(4):
        emit_pv(14, qp, 1)
        emit_pv(15, qp, 1)
        emit_o_chunk(7, (2 * qp, 2 * qp + 1))

    o_pool["pool"].release()
    attnp.release()
    psum.release()
    persist.release()


def _build():
    nc = bacc.Bacc("TRN2", target_bir_lowering=False, debug=False,
                   num_devices=NCORES)
    io = {}

    def inp(name, shape, dtype=BF):
        io[name] = nc.dram_tensor(name, shape, dtype, kind="ExternalInput").ap()
    inp("qT", [H, TOK])
    inp("kT", [H, KTOK])
    inp("vT", [H, KTOK])
    inp("wqT", [H, H])
    inp("wkT", [H, H])
    inp("wvT", [H, VW])
    inp("woT", [H, H])
    inp("bvb", [128, VW])
    inp("bob", [128, H])
    inp("bqcol", [128, IC], F32)
    inp("bkcol", [128, IC], F32)
    inp("maskcol", [128, KC], F32)
    io["out"] = nc.dram_tensor("out", [TOK, H], F32, kind="ExternalOutput").ap()

    with tile.TileContext(nc) as tc:
        _emit(nc, tc, io)
    nc.compile()
    return nc, io


def get_compiled():
    if "nc" not in _CACHE:
        _CACHE["nc"], _CACHE["io"] = _build()
    return _CACHE["nc"]


def make_in_maps(query, key_, value, attention_mask, Wq, bq, Wk, bk, Wv, bv,
                 Wo, bo):
    bf = ml_dtypes.bfloat16
    f32 = np.float32
    query = np.asarray(query, f32)
    key_ = np.asarray(key_, f32)
    value = np.asarray(value, f32)
    attention_mask = np.asarray(attention_mask, f32)
    Wq, bq = np.asarray(Wq, f32), np.asarray(bq, f32)
    Wk, bk = np.asarray(Wk, f32), np.asarray(bk, f32)
    Wv, bv = np.asarray(Wv, f32), np.asarray(bv, f32)
    Wo, bo = np.asarray(Wo, f32), np.asarray(bo, f32)

    scale = 1.0 / np.sqrt(np.float32(HDIM))
    wqT = np.ascontiguousarray((Wq * scale).T).astype(bf)
    wkT = np.ascontiguousarray(Wk.T).astype(bf)
    woT = np.ascontiguousarray(Wo.T).astype(bf)
    wvT = np.zeros((H, VW), f32)
    bv_ext = np.zeros((1, VW), f32)
    for h in range(NH):
        wvT[:, h * 65:h * 65 + 64] = Wv[h * 64:(h + 1) * 64, :].T
        bv_ext[0, h * 65:h * 65 + 64] = bv[h * 64:(h + 1) * 64]
        bv_ext[0, h * 65 + 64] = 1.0
    wvT = wvT.astype(bf)
    bvb = np.broadcast_to(bv_ext, (128, VW)).astype(bf)
    bob = np.broadcast_to(bo.reshape(1, H), (128, H)).astype(bf)
    bqcol = np.ascontiguousarray((bq * scale).reshape(IC, 128).T).astype(f32)
    bkcol = np.ascontiguousarray(bk.reshape(IC, 128).T).astype(f32)

    in_maps = []
    for c in range(NCORES):
        b, half = divmod(c, 2)
        sl = slice(half * TOK, (half + 1) * TOK)
        qT = np.ascontiguousarray(query[b, sl, :].T).astype(bf)
        kT = np.ascontiguousarray(key_[b].T).astype(bf)
        vT = np.ascontiguousarray(value[b].T).astype(bf)
        # -2.0 shift guards fp8e4 overflow in exp(); softmax invariance
        # makes it exact.
        maskcol = np.ascontiguousarray(
            ((1.0 - attention_mask[b]) * -10000.0 - 2.0).reshape(KC, 128).T
        ).astype(f32)
        in_maps.append({
            "qT": qT, "kT": kT, "vT": vT,
            "wqT": wqT, "wkT": wkT, "wvT": wvT, "woT": woT,
            "bvb": bvb, "bob": bob,
            "bqcol": bqcol, "bkcol": bkcol,
            "maskcol": maskcol,
        })
    return in_maps


def kernel(query, key_, value, attention_mask, Wq, bq, Wk, bk, Wv, bv, Wo, bo,
           **run_kwargs):
    nc = get_compiled()
    in_maps = make_in_maps(query, key_, value, attention_mask, Wq, bq, Wk, bk,
                           Wv, bv, Wo, bo)
    res = run_bass_kernel_spmd(nc, in_maps, core_ids=list(range(NCORES)),
                               **run_kwargs)
    out = np.empty((B, S, H), np.float32)
    for c in range(NCORES):
        b, half = divmod(c, 2)
        out[b, half * TOK:(half + 1) * TOK, :] = res.results[c]["out"]
    if run_kwargs:
        kernel.last_results = res
    return out
